# revision 1
# baseline (speedup 1.0000x reference)
"""Causal attention (B=8, T=2048, D=1024, fp32 in/out) on 8 trn2 NeuronCores.

Sharding: data-parallel over batch — core b computes batch element b.
Host-side prep (part of kernel()): per-batch slices, Q^T / K^T
relayouts so the device receives d-major operands directly, and a cast
to bf16 (halves DMA bytes and removes the fp32r transpose penalty).

Per-core device kernel (flash-style, causal block-skipped):
  S[q,k] = QT.T @ KT    (TensorE, bf16 in, 512-wide f32 PSUM k-tiles)
  P      = exp((S + mask) * 1/sqrt(D))   (ScalarE, bf16 out)
  rowsum = reduce_sum(P)               (VectorE, f32)
  P^T    = PE transpose per 128-block  (the only on-device transposes)
  O      = (P^T.T @ V) / rowsum        (TensorE f32 PSUM accum, VectorE
                                        normalize, bf16 store)

The emission is software-pipelined ACROSS q-blocks: each block's final
P^T/O stage (which waits on the exp) is emitted after the NEXT block's
S matmuls, so the exp latency never bubbles the in-order PE stream.
HAM: dummy warm-up matmuls keep the PE clock at 8/8 through the initial
DMA window — any PE idle gap triggers a ~2x downclock for several us.
"""

import sys

if "/opt/trn_rl_repo" not in sys.path:
    sys.path.insert(0, "/opt/trn_rl_repo")

import numpy as np

B, T, D = 8, 2048, 1024
NQ = T // 128   # 16 query blocks of 128
NKC = T // 128  # 16 key chunks of 128
ND = D // 128   # 8 d chunks of 128
KTW = 512       # key tile width for S (one 2KB f32 PSUM bank)
KCPT = KTW // 128  # key chunks per S tile
QPB = 4         # q-blocks per resident Q^T pass
NEG = -1e10
SOFTMAX_SCALE = 1.0 / float(np.sqrt(D))

_CACHE = {}


def _split_waits(nc):
    """This container's walrus accepts only ONE sync-wait per instruction
    (setupSyncWait: 'Too many sync wait commands').  Tile freely attaches
    several waits to one instruction.  Hoist the extras onto same-engine
    NoOps inserted immediately before the instruction — each engine
    executes its stream in order, so the wait semantics are unchanged."""
    import concourse.mybir as mybir

    n_split = 0
    for f in nc.m.functions:
        for bb in f.blocks:
            out = []
            for inst in bb.instructions:
                si = inst.sync_info
                if si is not None and len(si.on_wait) > 1:
                    waits = list(si.on_wait)
                    for w in waits[:-1]:
                        nop = mybir.InstNoOp(
                            name=f"{inst.name}-w{n_split}",
                            engine=inst.engine,
                            sync_info=mybir.SyncInfo(on_wait=[w], on_update=[]),
                            bass_nofuse=True,
                        )
                        out.append(nop)
                        n_split += 1
                    inst.sync_info = mybir.SyncInfo(
                        on_wait=[waits[-1]], on_update=list(si.on_update)
                    )
                out.append(inst)
            bb.instructions[:] = out
    return n_split


def _build():
    import concourse.bass as bass
    import concourse.mybir as mybir
    import concourse.tile as tile
    from concourse import masks

    f32 = mybir.dt.float32
    bf16 = mybir.dt.bfloat16
    EXP = mybir.ActivationFunctionType.Exp
    X = mybir.AxisListType.X

    nc = bass.Bass()
    # Q^T / K^T arrive pre-swizzled from the host as [p, slice, dc, c]
    # (slice = 512 t-cols), so every input DMA slice is contiguous on BOTH
    # the dram and SBUF side: 128 descriptors of 8KB instead of 1024 of
    # 1KB — ~2x faster descriptor generation and better wire efficiency.
    qt_d = nc.dram_tensor("query_t", [128, 4, ND, 512], bf16,
                          kind="ExternalInput")
    kt_d = nc.dram_tensor("key_t", [128, 4, ND, 512], bf16,
                          kind="ExternalInput")
    v_d = nc.dram_tensor("value", [T, D], bf16, kind="ExternalInput")
    o_d = nc.dram_tensor("out", [T, D], bf16, kind="ExternalOutput")

    with tile.TileContext(nc) as tc:
        with (
            tc.tile_pool(name="const", bufs=1) as constp,
            tc.tile_pool(name="big", bufs=1) as bigp,
            tc.tile_pool(name="qtpool", bufs=2) as qtpool,
            tc.tile_pool(name="p", bufs=3) as pp,
            tc.tile_pool(name="pt", bufs=2) as ptp,
            tc.tile_pool(name="osb", bufs=2) as osbp,
            tc.tile_pool(name="small", bufs=2) as smallp,
            tc.tile_pool(name="psum_s", bufs=3, space="PSUM") as psum_s,
            tc.tile_pool(name="psum_tr", bufs=1, space="PSUM") as psum_tr,
            tc.tile_pool(name="psum_o", bufs=2, space="PSUM") as psum_o,
        ):
            ident_f = constp.tile([128, 128], f32)
            masks.make_identity(nc, ident_f[:])
            ident = constp.tile([128, 128], bf16)
            nc.vector.tensor_copy(ident[:], ident_f[:])
            ident_r = ident[:]

            # Causal additive mask for the diagonal 128x128 block: with
            # exact-width diagonal k-tiles only the LAST 128 columns ever
            # need masking, always with the same strict upper triangle:
            # cmask[p, c] = 0 if c <= p else NEG
            cmask = constp.tile([128, 128], f32)
            nc.gpsimd.memset(cmask[:], 0.0)
            nc.gpsimd.affine_select(
                out=cmask[:],
                in_=cmask[:],
                compare_op=mybir.AluOpType.is_ge,
                fill=NEG,
                base=0,
                channel_multiplier=1,
                pattern=[[-1, 128]],
            )

            v_all = bigp.tile([128, NKC, D], bf16)
            kt_all = bigp.tile([128, 4, ND, 512], bf16)

            # HAM heater: PE is otherwise idle during the first DMAs; a burst
            # of dummy matmuls flips the clock gate to 8/8 before real work.
            # Sized to end right as the first Q^T/K^T transfers land.
            heat_src = constp.tile([128, 128], f32)
            nc.vector.memset(heat_src[:], 1.0)
            heat_ps = psum_o.tile([128, D], f32, tag="ops", name="heat")
            for _ in range(23):
                nc.tensor.matmul(heat_ps[:, :128], heat_src[:], heat_src[:],
                                 start=True, stop=True)

            # ---- DMA plumbing --------------------------------------------
            # Q^T arrives in passes of QPB q-blocks (double-buffered);
            # K^T arrives in 512-column slices across all d-chunks so early
            # q-blocks unblock quickly; V streams per key chunk.  All on the
            # Sync HWDGE queue, issued lazily: concurrent early transfers
            # steal DMA bandwidth from the critical first loads.
            QT_PASSES = list(range(0, NQ, QPB))
            qt_tiles = {}
            qt_for = {}

            def issue_qt_pass(pi):
                if pi < len(QT_PASSES) and pi not in qt_tiles:
                    s = QT_PASSES[pi]
                    qt = qtpool.tile([128, ND, QPB * 128], bf16, tag="qtpass",
                                     name=f"qtp{pi}")
                    nc.sync.dma_start(qt[:], qt_d[:, pi])
                    qt_tiles[pi] = qt
                    for j in range(QPB):
                        qt_for[s + j] = (qt, j * 128)

            def issue_kt_slice(si):
                nc.sync.dma_start(kt_all[:, si], kt_d[:, si])

            # ---- per-q-block stage emitters ------------------------------
            state = {}

            def n_kt_of(qb):
                return (qb + KCPT) // KCPT

            def emit_alloc(qb):
                st = state[qb] = {}
                st["asum"] = smallp.tile([128, 4], f32, tag="asum",
                                         name=f"asum{qb}")
                st["o_ps"] = psum_o.tile([128, D], f32, tag="ops",
                                         name=f"ops{qb}")
                st["p"] = {}

            def emit_qkt(qb, kt):
                st = state[qb]
                n_kc = qb + 1
                n_kt = n_kt_of(qb)
                qt, qoff = qt_for[qb]
                # diagonal k-tile: shrink to the exact 128-multiple of kspan
                if kt == n_kt - 1:
                    rem = n_kc - kt * KCPT  # 1..KCPT chunks of 128
                    w = rem * 128
                else:
                    w = KTW
                s_ps = psum_s.tile([128, KTW], f32, tag="s_ps")
                for dc in range(ND):
                    nc.tensor.matmul(
                        s_ps[:, :w],
                        qt[:, dc, qoff:qoff + 128],
                        kt_all[:, kt, dc, :w],
                        start=(dc == 0),
                        stop=(dc == ND - 1),
                    )
                if kt == n_kt - 1:
                    # only the diagonal 128x128 block needs the triangle mask
                    nc.vector.tensor_add(
                        s_ps[:, w - 128:w], s_ps[:, w - 128:w], cmask[:]
                    )
                p_sb = pp.tile([128, KTW], bf16)
                nc.scalar.activation(
                    p_sb[:, :w], s_ps[:, :w], EXP,
                    bias=0.0, scale=SOFTMAX_SCALE,
                )
                # row-sums on DVE instead of ACT accum_out: saves the 285ns
                # ACTIVATION_READ_ACCUMULATOR between exps on the Scalar
                # queue (the exp chain paces the tail cascade)
                nc.vector.reduce_sum(
                    st["asum"][:, kt:kt + 1], p_sb[:, :w], axis=X
                )
                st["p"][kt] = p_sb

            def emit_ptpv(qb, kt, halves=None):
                st = state[qb]
                n_kc = qb + 1
                p_sb = st["p"].pop(kt)
                o_ps = st["o_ps"]
                n_j = min(KCPT, n_kc - kt * KCPT)
                pt_ps = psum_tr.tile([128, KTW], bf16, tag="tr")
                pt_sb = ptp.tile([128, KTW], bf16)
                for j in range(n_j):
                    nc.tensor.transpose(
                        pt_ps[:, j * 128:(j + 1) * 128],
                        p_sb[:, j * 128:(j + 1) * 128],
                        ident_r,
                    )
                nc.vector.tensor_copy(pt_sb[:, :n_j * 128], pt_ps[:, :n_j * 128])
                st["pt"] = pt_sb
                if halves is None:
                    halves = (0, 1)
                for h in halves:
                    for j in range(n_j):
                        kc = kt * KCPT + j
                        nc.tensor.matmul(
                            o_ps[:, h * 512:(h + 1) * 512],
                            pt_sb[:, j * 128:(j + 1) * 128],
                            v_all[:, kc, h * 512:(h + 1) * 512],
                            start=(kc == 0),
                            stop=(kc == n_kc - 1),
                        )

            def emit_rsum(qb):
                st = state[qb]
                n_kt = n_kt_of(qb)
                rsum = smallp.tile([128, 1], f32, tag="rsum")
                nc.vector.reduce_sum(rsum[:], st["asum"][:, :n_kt], axis=X)
                rinv = smallp.tile([128, 1], f32, tag="rinv")
                nc.vector.reciprocal(rinv[:], rsum[:])
                st["rinv"] = rinv

            def emit_drain(qb):
                """Final P^T/O stage + normalize + store for a finished block."""
                emit_ptpv(qb, n_kt_of(qb) - 1)
                emit_rsum(qb)
                st = state[qb]
                o_sb = osbp.tile([128, D], bf16)
                nc.vector.tensor_scalar_mul(
                    o_sb[:, 0:512], st["o_ps"][:, 0:512], st["rinv"][:])
                nc.vector.tensor_scalar_mul(
                    o_sb[:, 512:1024], st["o_ps"][:, 512:1024], st["rinv"][:])
                nc.sync.dma_start(o_d[qb * 128:(qb + 1) * 128, :], o_sb[:])
                state.pop(qb)

            def emit_drain_final(qb):
                """Tail variant: finish PSUM half 0 and ship it while half
                1's matmuls still run."""
                st = state[qb]
                n_kc = qb + 1
                kt = n_kt_of(qb) - 1
                emit_rsum(qb)
                emit_ptpv(qb, kt, halves=(0,))
                o_sb = osbp.tile([128, D], bf16)
                nc.vector.tensor_scalar_mul(
                    o_sb[:, 0:512], st["o_ps"][:, 0:512], st["rinv"][:])
                nc.sync.dma_start(o_d[qb * 128:(qb + 1) * 128, 0:512],
                                  o_sb[:, 0:512])
                pt_sb = st["pt"]
                n_j = min(KCPT, n_kc - kt * KCPT)
                for j in range(n_j):
                    kc = kt * KCPT + j
                    nc.tensor.matmul(
                        st["o_ps"][:, 512:1024],
                        pt_sb[:, j * 128:(j + 1) * 128],
                        v_all[:, kc, 512:1024],
                        start=(kc == 0),
                        stop=(kc == n_kc - 1),
                    )
                nc.vector.tensor_scalar_mul(
                    o_sb[:, 512:1024], st["o_ps"][:, 512:1024], st["rinv"][:])
                # terminal store via the Scalar HWDGE queue (idle once the
                # exps are done) so its issue overlaps half 0's on Sync
                nc.scalar.dma_start(o_d[qb * 128:(qb + 1) * 128, 512:1024],
                                    o_sb[:, 512:1024])
                state.pop(qb)

            # ---- merged, cross-block-pipelined emission ------------------
            pending = []  # blocks whose final ptpv+fin is deferred

            def emit_qblock(qb):
                emit_alloc(qb)
                n_kt = n_kt_of(qb)
                for kt in range(n_kt):
                    emit_qkt(qb, kt)
                    if kt == 0 and pending:
                        emit_drain(pending.pop())
                    if kt >= 1:
                        emit_ptpv(qb, kt - 1)
                if pending:
                    emit_drain(pending.pop())
                pending.append(qb)

            issue_qt_pass(0)
            issue_kt_slice(0)
            nc.sync.dma_start(v_all[:, 0, :], v_d[0:128, :])
            issue_qt_pass(1)
            for kc in range(1, NKC):
                if kc <= 3:
                    issue_kt_slice(kc)
                nc.sync.dma_start(
                    v_all[:, kc, :], v_d[kc * 128:(kc + 1) * 128, :]
                )
                emit_qblock(kc - 1)
                if kc in (5, 9):
                    # prefetch the next Q^T pass right after the last reader
                    # of the pass whose slot it reclaims has been emitted
                    issue_qt_pass({5: 2, 9: 3}[kc])
            emit_qblock(NQ - 1)
            emit_drain_final(pending.pop())

    _split_waits(nc)
    return nc


def _np_reference(query, key, value, mask):
    """Host fallback for the general (non-all-ones) padding-mask case."""
    out = np.empty_like(query)
    tri = np.triu(np.ones((T, T), dtype=np.float32), 1) * 1e10
    for b in range(B):
        s = query[b] @ key[b].T
        s = s - tri
        s = s - (1.0 - mask[b])[None, :] * 1e10
        s = s * SOFTMAX_SCALE
        s = s - s.max(axis=-1, keepdims=True)
        p = np.exp(s)
        p = p / p.sum(axis=-1, keepdims=True)
        out[b] = p @ value[b]
    return out


def make_in_maps(query, key, value):
    """Per-core input dicts: batch b -> core b.

    Q^T / K^T are swizzled host-side to [p, slice, dc, c] (slice = 512
    t-cols, dc = 128-wide d-chunk) so each device DMA slice is one
    contiguous 8KB run per partition on both the dram and SBUF side.
    """
    import ml_dtypes

    bf = ml_dtypes.bfloat16

    def swizzle(x):  # [T, D] -> [p, slice, dc, c]
        return np.ascontiguousarray(
            x.reshape(4, 512, ND, 128).transpose(3, 0, 2, 1)
        ).astype(bf)

    maps = []
    for b in range(B):
        maps.append({
            "query_t": swizzle(query[b]),
            "key_t": swizzle(key[b]),
            "value": np.ascontiguousarray(value[b]).astype(bf),
        })
    return maps


def kernel(query, key, value, mask):
    query = np.asarray(query, dtype=np.float32)
    key = np.asarray(key, dtype=np.float32)
    value = np.asarray(value, dtype=np.float32)
    mask = np.asarray(mask, dtype=np.float32)

    if not np.all(mask == 1.0):
        return _np_reference(query, key, value, mask)

    from concourse.bass_utils import run_bass_kernel_spmd

    if "nc" not in _CACHE:
        _CACHE["nc"] = _build()
    nc = _CACHE["nc"]

    in_maps = make_in_maps(query, key, value)
    last_err = None
    for _ in range(3):  # retry transient device errors (NRT_EXEC_UNIT_...)
        try:
            res = run_bass_kernel_spmd(nc, in_maps, core_ids=list(range(B)))
            break
        except Exception as e:  # noqa: BLE001
            last_err = e
    else:
        raise last_err
    out = np.stack([res.results[b]["out"] for b in range(B)], axis=0)
    return out.astype(np.float32)

